# revision 27
# baseline (speedup 1.0000x reference)
"""Trainium2 Bass kernel for nn_Mlpmoe (moe_routing).

Structure of the problem (B=64, P=256, D=768, H=3072, 6 classes, 5+5 expert atoms):
  - patch tokens [B,256,D] go through a dense MLP (W1 -> gelu -> W2)   (~155 GFLOP)
  - 6 cls tokens  [B,6,D] each go through 2 experts (atom1 -> gelu -> atom2),
    combined with a top-1 softmax gate over 2 experts per class         (~7 GFLOP)

Sharding over 8 NeuronCores:
  - patch MLP: data-parallel over batch (8 batches/core), moe0 weights replicated,
    computed in bf16 with fp32 PSUM accumulation.
  - cls experts: hidden-dim (H) parallel — core c computes all classes/batches for
    H-slice [c*384,(c+1)*384) of every atom; per-core partial outputs (already
    multiplied by the 0/1 gate weights) are summed on the host.
  - gate weights g (exactly 0.0/1.0 after top-1 + renorm) are computed on host
    from fp32 logits (min observed logit margin ~1.7e-3 >> fp32 noise) and fed
    to the device as a broadcast multiplier.

All activations/weights are fed to the device in bf16 (layouts pre-transposed on
host so no on-device transposes are needed); outputs come back fp32.
"""

import numpy as np
import ml_dtypes

# ---------------------------------------------------------------- constants
NCORES = 8
B, PT, D, H = 64, 256, 768, 3072
NCLS = 6
KD = D // 128            # 6 contraction tiles of 128 over D
MH = H // 128            # 24 tiles over H
HS = H // NCORES         # 384 per-core hidden slice
HK = HS // 128           # 3 tiles over the slice
BPC = B // NCORES        # 8 batches per core
TPC = BPC * PT           # 2048 patch tokens per core
TN = 512                 # token tile (matmul free dim / one PSUM bank)
NT = TPC // TN           # 4 token tiles

PAIRS = [[(0, 3), (3, 0)], [(0, 4), (4, 0)], [(1, 3), (3, 1)],
         [(1, 4), (4, 1)], [(2, 3), (3, 2)], [(2, 4), (4, 2)]]
# classes using atom j (same list for atom1 and atom2 roles)
CLASSES_OF = [[0, 1], [2, 3], [4, 5], [0, 2, 4], [1, 3, 5]]
GW = [64 * len(c) for c in CLASSES_OF]          # group widths: 128,128,128,192,192
GOFF = np.cumsum([0] + GW).tolist()             # 0,128,256,384,576,(768)
APP0 = np.cumsum([0] + [len(c) for c in CLASSES_OF]).tolist()  # app offsets in gb


def _other_atom(i, j):
    a, c = PAIRS[i][0]
    return c if j == a else a


_NC_CACHE = {}


def _build_nc():
    """Build + bacc-compile the (SPMD, identical on all cores) Bass program."""
    if "nc" in _NC_CACHE:
        return _NC_CACHE["nc"]

    from contextlib import ExitStack
    import concourse.bass as bass  # noqa: F401
    import concourse.mybir as mybir
    import concourse.tile as tile
    from concourse import bacc
    from concourse.tile import add_dep_helper

    f32 = mybir.dt.float32
    bf16 = mybir.dt.bfloat16
    AF = mybir.ActivationFunctionType
    ts = bass.ts

    # disable_frame_to_traceback keeps python source paths out of the BIR, so
    # the compiled-NEFF cache hits no matter which directory kernel.py runs from
    nc = bacc.Bacc("TRN2", target_bir_lowering=False, debug=False,
                   enable_asserts=False, num_devices=NCORES,
                   disable_frame_to_traceback=True)

    xp = nc.dram_tensor("xp", [128, KD, TPC], bf16, kind="ExternalInput").ap()
    # hm-major weight layout: [p, hm, k, 128] so the first L1 psum group only
    # needs the first small chunk of W1 before matmuls can start
    w1 = nc.dram_tensor("w1", [128, MH, KD, 128], bf16, kind="ExternalInput").ap()
    w2 = nc.dram_tensor("w2", [128, MH, D], bf16, kind="ExternalInput").ap()
    b1 = nc.dram_tensor("b1", [128, MH], f32, kind="ExternalInput").ap()
    b2 = nc.dram_tensor("b2", [128, KD], f32, kind="ExternalInput").ap()
    xc = nc.dram_tensor("xc", [128, KD, 768], bf16, kind="ExternalInput").ap()
    a1 = nc.dram_tensor("a1", [5, 128, KD, HS], bf16, kind="ExternalInput").ap()
    a1b = nc.dram_tensor("a1b", [128, 5, HK], f32, kind="ExternalInput").ap()
    a2 = nc.dram_tensor("a2", [5, 128, HK, D], bf16, kind="ExternalInput").ap()
    gb = nc.dram_tensor("gb", [128, sum(len(c) for c in CLASSES_OF), 64], f32,
                        kind="ExternalInput").ap()
    yp = nc.dram_tensor("yp", [128, KD, TPC], f32, kind="ExternalOutput").ap()
    yc = nc.dram_tensor("yc", [128, KD, NCLS * 64], f32, kind="ExternalOutput").ap()

    with tile.TileContext(nc) as tc, ExitStack() as ctx:
        wp = ctx.enter_context(tc.tile_pool(name="weights", bufs=1))
        xpool = ctx.enter_context(tc.tile_pool(name="xin", bufs=2))
        hpool = ctx.enter_context(tc.tile_pool(name="hmid", bufs=1))
        opool = ctx.enter_context(tc.tile_pool(name="out", bufs=1))
        cpool = ctx.enter_context(tc.tile_pool(name="cls", bufs=1))
        tpool = ctx.enter_context(tc.tile_pool(name="tmp", bufs=2))
        pspool = ctx.enter_context(tc.tile_pool(name="ps", bufs=8, space="PSUM"))

        # ---- DMA schedule ------------------------------------------------
        # critical path (sync HWDGE ring): a small first W1 chunk + first
        # token tile so matmuls start ASAP, then the rest of W1.
        w1t = wp.tile([128, MH, KD, 128], bf16)
        nc.sync.dma_start(w1t[:, 0:6], w1[:, 0:6])
        xt0 = xpool.tile([128, KD, TN], bf16, tag="xt", name="xt0")
        nc.sync.dma_start(xt0[:], xp[:, :, ts(0, TN)])
        b1t = wp.tile([128, MH], f32)
        nc.sync.dma_start(b1t[:], b1[:])
        b2t = wp.tile([128, KD], f32)
        nc.sync.dma_start(b2t[:], b2[:])

        # non-critical loads go on the scalar HWDGE ring, triggered between
        # gelu activations so they don't steal DMA bandwidth from W1/x0
        w2t = wp.tile([128, MH, D], bf16)
        xct = cpool.tile([128, KD, 768], bf16)
        a1t = cpool.tile([128, 5, KD, HS], bf16)
        a1bt = cpool.tile([128, 5, HK], f32)
        a2t = cpool.tile([128, 5, HK, D], bf16)
        gbt = cpool.tile([128, APP0[-1], 64], f32)
        h2 = [cpool.tile([128, HK, GW[c]], bf16, tag=f"h2_{c}", name=f"h2_{c}")
              for c in range(5)]
        outc = cpool.tile([128, KD, NCLS * 64], f32)

        # second token tile preallocated so its load can be deferred (with
        # bufs=2 the slot is free at kernel start, so an in-loop load would be
        # hoisted into the startup window and steal bandwidth from W1/x0)
        xt1 = xpool.tile([128, KD, TN], bf16, tag="xt", name="xt1")

        # one trigger per gelu slot: remaining W1 chunks stream just ahead of
        # the consuming psum groups, W2 by t0-L2 (~45us), cls inputs by ~220us
        _dl = {}
        _dl[(0, 13)] = lambda: nc.scalar.dma_start(xt1[:], xp[:, :, ts(1, TN)])
        for i, hm in enumerate(range(0, 18, 2)):
            _dl[(0, hm)] = lambda i=i: nc.scalar.dma_start(
                w1t[:, 6 + 2 * i:8 + 2 * i], w1[:, 6 + 2 * i:8 + 2 * i])
        for i in range(4):
            _dl[(0, 17 + 2 * i)] = lambda i=i: nc.scalar.dma_start(
                w2t[:, 6 * i:6 * (i + 1)], w2[:, 6 * i:6 * (i + 1)])
        _dl[(1, 0)] = lambda: nc.scalar.dma_start(xct[:], xc[:])
        for j in range(5):
            _dl[(1, 2 + 2 * j)] = lambda j=j: nc.scalar.dma_start(a1t[:, j], a1[j])
            _dl[(1, 14 + 2 * j)] = lambda j=j: nc.scalar.dma_start(a2t[:, j], a2[j])
        _dl[(1, 12)] = lambda: nc.scalar.dma_start(a1bt[:], a1b[:])
        _dl[(2, 0)] = lambda: nc.scalar.dma_start(gbt[:], gb[:])

        def _ins(x):
            return getattr(x, "ins", x)

        def deferred_loads(t, hm, act):
            fn = _dl.get((t, hm))
            if fn is not None:
                d = fn()
                # same-engine ordering edge: without it the scheduler hoists
                # the (dependency-free) trigger to kernel start, where its
                # transfer steals DMA bandwidth from the critical W1/x0 loads
                add_dep_helper(_ins(d), _ins(act), sync=False,
                               reason="defer bulk load behind gelu")

        # ---- patch MLP tile body ----------------------------------------
        def patch_tile(t, xt):
            ht = hpool.tile([128, MH, TN], bf16, tag="ht", name="ht")
            for hm in range(MH):
                pt = pspool.tile([128, TN], f32, tag="ps", name="pt")
                for k in range(KD):
                    nc.tensor.matmul(pt[:], w1t[:, hm, k, :], xt[:, k, :],
                                     start=(k == 0), stop=(k == KD - 1))
                act = nc.scalar.activation(ht[:, hm, :], pt[:], AF.Gelu,
                                           bias=b1t[:, hm, None])
                deferred_loads(t, hm, act)
            ot = opool.tile([128, KD, TN], f32, tag="ot", name="ot")
            for dm in range(KD):
                pt = pspool.tile([128, TN], f32, tag="ps", name="pt")
                for k in range(MH):
                    nc.tensor.matmul(pt[:], w2t[:, k, ts(dm, 128)], ht[:, k, :],
                                     start=(k == 0), stop=(k == MH - 1))
                nc.vector.tensor_scalar_add(ot[:, dm, :], pt[:], b2t[:, dm, None])
                # per-dm store so the tail only waits for the last 256 KB
                nc.sync.dma_start(yp[:, dm, ts(t, TN)], ot[:, dm, :])

        def cls_phase():
            # first layer: per atom1 group, gelu results scattered into the
            # atom2-grouped h2 buffers
            for j in range(5):
                for hm in range(HK):
                    pt = pspool.tile([128, TN], f32, tag="ps", name="pt")
                    po = pt[:, :GW[j]]
                    for k in range(KD):
                        nc.tensor.matmul(po, a1t[:, j, k, ts(hm, 128)],
                                         xct[:, k, GOFF[j]:GOFF[j] + GW[j]],
                                         start=(k == 0), stop=(k == KD - 1))
                    for bidx, i in enumerate(CLASSES_OF[j]):
                        cdst = _other_atom(i, j)
                        pos = CLASSES_OF[cdst].index(i)
                        nc.scalar.activation(h2[cdst][:, hm, ts(pos, 64)],
                                             po[:, ts(bidx, 64)], AF.Gelu,
                                             bias=a1bt[:, j, hm, None])

            # second layer: per atom2 group, gated write/accumulate into outc
            outc4 = outc[:].rearrange("p d (l t) -> p d l t", t=64)
            for c in range(5):
                L = len(CLASSES_OF[c])
                gsl = gbt[:, APP0[c]:APP0[c] + L, :]
                for dm in range(KD):
                    pt = pspool.tile([128, TN], f32, tag="ps", name="pt")
                    po = pt[:, :GW[c]]
                    for hk in range(HK):
                        nc.tensor.matmul(po, a2t[:, c, hk, ts(dm, 128)],
                                         h2[c][:, hk, :],
                                         start=(hk == 0), stop=(hk == HK - 1))
                    pv = po.rearrange("p (l t) -> p l t", t=64)
                    if c < 3:
                        # classes 2c,2c+1 - first write of those columns
                        dst = outc4[:, dm, 2 * c:2 * c + 2, :]
                        nc.vector.tensor_tensor(dst, pv, gsl,
                                                mybir.AluOpType.mult)
                    else:
                        tmp = tpool.tile([128, 3, 64], f32, tag="ctmp",
                                         name="ctmp")
                        nc.vector.tensor_tensor(tmp[:, :L, :], pv, gsl,
                                                mybir.AluOpType.mult)
                        dst = outc4[:, dm, c - 3:NCLS:2, :]
                        nc.vector.tensor_add(dst, dst, tmp[:, :L, :])
            nc.sync.dma_start(yc[:], outc[:])

        # PE warmup: the HAM clock gate keeps the PE at 1.2 GHz until it has
        # been busy ~3.4us. The critical W1/x0 DMA takes ~8us, so run dummy
        # matmuls on zeros during that window - the real stream then starts
        # at 2.4 GHz instead of paying ~13 cold matmuls.
        warm = wp.tile([128, TN], bf16)
        nc.gpsimd.memset(warm[:], 0.0)
        wps = pspool.tile([128, TN], f32, tag="ps", name="warmps")
        for _ in range(14):
            nc.tensor.matmul(wps[:], warm[:, :128], warm[:], start=True, stop=True)

        # patch tiles 0..2, then cls (its epilogue hides under tile 3)
        for t in range(NT - 1):
            if t == 0:
                xt = xt0
            elif t == 1:
                xt = xt1  # load deferred to a t0 gelu slot
            else:
                xt = xpool.tile([128, KD, TN], bf16, tag="xt", name="xt")
                nc.sync.dma_start(xt[:], xp[:, :, ts(t, TN)])
            patch_tile(t, xt)
        cls_phase()
        xt = xpool.tile([128, KD, TN], bf16, tag="xt", name="xt")
        nc.sync.dma_start(xt[:], xp[:, :, ts(NT - 1, TN)])
        patch_tile(NT - 1, xt)

    nc.compile()
    _NC_CACHE["nc"] = nc
    return nc


# ---------------------------------------------------------------- host glue
def _bf(a):
    return np.ascontiguousarray(np.asarray(a), dtype=ml_dtypes.bfloat16)


def _f32(a):
    return np.ascontiguousarray(np.asarray(a), dtype=np.float32)


def _gates(x, G_W):
    """Mirror the reference's softmax/top-1/renorm gating in fp32 on host."""
    cls_tokens = np.asarray(x[:, :NCLS], dtype=np.float32)
    logits = np.einsum("bid,ide->bie", cls_tokens, np.asarray(G_W, np.float32))
    m = logits.max(-1, keepdims=True)
    e = np.exp(logits - m)
    gate = e / e.sum(-1, keepdims=True)
    thr = np.sort(gate, axis=-1)[..., -2]
    mask = (gate > thr[..., None]).astype(np.float32)
    g = gate * mask
    g = g / np.clip(g.sum(-1, keepdims=True), 1e-6, None)
    return g  # [B, NCLS, 2], entries exactly 0.0 or 1.0 (or 0/0 on exact ties)


def _shard_inputs(x, moe0_W1, moe0_b1, moe0_W2, moe0_b2, A1_W, A1_b, A2_W, A2_b, G_W):
    x = np.asarray(x, np.float32)

    # shared (replicated) tensors
    # [d, h] -> [p, hm, k, c] with d = k*128+p, h = hm*128+c
    w1v = _bf(np.asarray(moe0_W1, np.float32)).reshape(KD, 128, MH, 128)
    w1v = np.ascontiguousarray(w1v.transpose(1, 2, 0, 3))
    w2v = _bf(np.asarray(moe0_W2, np.float32)).reshape(MH, 128, D).transpose(1, 0, 2)
    w2v = np.ascontiguousarray(w2v)
    b1v = np.ascontiguousarray(_f32(moe0_b1).reshape(MH, 128).T)
    b2v = np.ascontiguousarray(_f32(moe0_b2).reshape(KD, 128).T)

    # stacked cls token groups -> [128, KD, 768] bf16
    xc_f = x[:, :NCLS, :]                                   # [B, 6, D]
    stacked = np.concatenate([xc_f[:, i, :] for j in range(5)
                              for i in CLASSES_OF[j]], axis=0)  # [768, D]
    xcv = _bf(stacked.T.reshape(KD, 128, 768).transpose(1, 0, 2))

    # gate multiplier broadcast [128, 12, 64]
    g = _gates(x, G_W)
    g_app = np.empty((APP0[-1], B), np.float32)
    for c in range(5):
        for pos, i in enumerate(CLASSES_OF[c]):
            e = [p[1] for p in PAIRS[i]].index(c)
            g_app[APP0[c] + pos] = g[:, i, e]
    gbv = np.ascontiguousarray(np.broadcast_to(g_app[None], (128,) + g_app.shape))

    A1_W = np.asarray(A1_W, np.float32)
    A2_W = np.asarray(A2_W, np.float32)
    A1_b = np.asarray(A1_b, np.float32)

    in_maps = []
    for core in range(NCORES):
        hs = slice(core * HS, (core + 1) * HS)
        # per-core patch tokens, transposed: [128, KD, TPC]
        xpc = x[core * BPC:(core + 1) * BPC, NCLS:, :].reshape(TPC, D)
        xpv = _bf(xpc.T.reshape(KD, 128, TPC).transpose(1, 0, 2))
        # atom slices
        a1v = _bf(A1_W[:, :, hs].reshape(5, KD, 128, HS).transpose(0, 2, 1, 3))
        a2v = _bf(A2_W[:, hs, :].reshape(5, HK, 128, D).transpose(0, 2, 1, 3))
        a1bv = np.ascontiguousarray(
            A1_b[:, hs].reshape(5, HK, 128).transpose(2, 0, 1))
        in_maps.append({
            "xp": xpv, "w1": w1v, "w2": w2v, "b1": b1v, "b2": b2v,
            "xc": xcv, "a1": a1v, "a1b": a1bv, "a2": a2v, "gb": gbv,
        })
    return in_maps, g


def _combine_outputs(results, g, A2_b):
    A2_b = np.asarray(A2_b, np.float32)
    out = np.empty((B, NCLS + PT, D), np.float32)
    for core in range(NCORES):
        ypv = results[core]["yp"]  # [128, KD, TPC]
        out[core * BPC:(core + 1) * BPC, NCLS:, :] = (
            ypv.transpose(2, 1, 0).reshape(BPC, PT, D))

    ycs = np.zeros((128, KD, NCLS, 64), np.float64)
    for core in range(NCORES):
        ycs += results[core]["yc"].reshape(128, KD, NCLS, 64)
    cls_out = ycs.transpose(3, 2, 1, 0).reshape(B, NCLS, D).astype(np.float32)

    # gated atom2 bias term (biases are added pre-gating in the reference)
    a2b_sel = np.stack([[A2_b[p[1]] for p in PAIRS[i]] for i in range(NCLS)])
    cls_out += np.einsum("bie,ied->bid", g, a2b_sel).astype(np.float32)
    out[:, :NCLS, :] = cls_out
    return out


def _run(inputs, trace=False, trace_kwargs=None):
    from concourse.bass_utils import run_bass_kernel_spmd

    nc = _build_nc()
    in_maps, g = _shard_inputs(
        inputs["x"], inputs["moe0_W1"], inputs["moe0_b1"], inputs["moe0_W2"],
        inputs["moe0_b2"], inputs["A1_W"], inputs["A1_b"], inputs["A2_W"],
        inputs["A2_b"], inputs["G_W"])
    res = run_bass_kernel_spmd(nc, in_maps, core_ids=list(range(NCORES)),
                               trace=trace, **(trace_kwargs or {}))
    out = _combine_outputs(res.results, g, inputs["A2_b"])
    return out, res


def kernel(**inputs) -> np.ndarray:
    out, _ = _run(inputs, trace=False)
    return out


# revision 28
# speedup vs baseline: 1.0072x; 1.0072x over previous
"""Trainium2 Bass kernel for nn_Mlpmoe (moe_routing).

Structure of the problem (B=64, P=256, D=768, H=3072, 6 classes, 5+5 expert atoms):
  - patch tokens [B,256,D] go through a dense MLP (W1 -> gelu -> W2)   (~155 GFLOP)
  - 6 cls tokens  [B,6,D] each go through 2 experts (atom1 -> gelu -> atom2),
    combined with a top-1 softmax gate over 2 experts per class         (~7 GFLOP)

Sharding over 8 NeuronCores:
  - patch MLP: data-parallel over batch (8 batches/core), moe0 weights replicated,
    computed in bf16 with fp32 PSUM accumulation.
  - cls experts: hidden-dim (H) parallel — core c computes all classes/batches for
    H-slice [c*384,(c+1)*384) of every atom; per-core partial outputs (already
    multiplied by the 0/1 gate weights) are summed on the host.
  - gate weights g (exactly 0.0/1.0 after top-1 + renorm) are computed on host
    from fp32 logits (min observed logit margin ~1.7e-3 >> fp32 noise) and fed
    to the device as a broadcast multiplier.

All activations/weights are fed to the device in bf16 (layouts pre-transposed on
host so no on-device transposes are needed); outputs come back fp32.
"""

import numpy as np
import ml_dtypes

# ---------------------------------------------------------------- constants
NCORES = 8
B, PT, D, H = 64, 256, 768, 3072
NCLS = 6
KD = D // 128            # 6 contraction tiles of 128 over D
MH = H // 128            # 24 tiles over H
HS = H // NCORES         # 384 per-core hidden slice
HK = HS // 128           # 3 tiles over the slice
BPC = B // NCORES        # 8 batches per core
TPC = BPC * PT           # 2048 patch tokens per core
TN = 512                 # token tile (matmul free dim / one PSUM bank)
NT = TPC // TN           # 4 token tiles

PAIRS = [[(0, 3), (3, 0)], [(0, 4), (4, 0)], [(1, 3), (3, 1)],
         [(1, 4), (4, 1)], [(2, 3), (3, 2)], [(2, 4), (4, 2)]]
# classes using atom j (same list for atom1 and atom2 roles)
CLASSES_OF = [[0, 1], [2, 3], [4, 5], [0, 2, 4], [1, 3, 5]]
GW = [64 * len(c) for c in CLASSES_OF]          # group widths: 128,128,128,192,192
GOFF = np.cumsum([0] + GW).tolist()             # 0,128,256,384,576,(768)
APP0 = np.cumsum([0] + [len(c) for c in CLASSES_OF]).tolist()  # app offsets in gb


def _other_atom(i, j):
    a, c = PAIRS[i][0]
    return c if j == a else a


_NC_CACHE = {}


def _build_nc():
    """Build + bacc-compile the (SPMD, identical on all cores) Bass program."""
    if "nc" in _NC_CACHE:
        return _NC_CACHE["nc"]

    from contextlib import ExitStack
    import concourse.bass as bass  # noqa: F401
    import concourse.mybir as mybir
    import concourse.tile as tile
    from concourse import bacc
    from concourse.tile import add_dep_helper

    f32 = mybir.dt.float32
    bf16 = mybir.dt.bfloat16
    AF = mybir.ActivationFunctionType
    ts = bass.ts

    # disable_frame_to_traceback keeps python source paths out of the BIR, so
    # the compiled-NEFF cache hits no matter which directory kernel.py runs from
    nc = bacc.Bacc("TRN2", target_bir_lowering=False, debug=False,
                   enable_asserts=False, num_devices=NCORES,
                   disable_frame_to_traceback=True)

    xp = nc.dram_tensor("xp", [128, KD, TPC], bf16, kind="ExternalInput").ap()
    # hm-major weight layout: [p, hm, k, 128] so the first L1 psum group only
    # needs the first small chunk of W1 before matmuls can start
    w1 = nc.dram_tensor("w1", [128, MH, KD, 128], bf16, kind="ExternalInput").ap()
    w2 = nc.dram_tensor("w2", [128, MH, D], bf16, kind="ExternalInput").ap()
    b1 = nc.dram_tensor("b1", [128, MH], f32, kind="ExternalInput").ap()
    b2 = nc.dram_tensor("b2", [128, KD], f32, kind="ExternalInput").ap()
    xc = nc.dram_tensor("xc", [128, KD, 768], bf16, kind="ExternalInput").ap()
    a1 = nc.dram_tensor("a1", [5, 128, KD, HS], bf16, kind="ExternalInput").ap()
    a1b = nc.dram_tensor("a1b", [128, 5, HK], f32, kind="ExternalInput").ap()
    a2 = nc.dram_tensor("a2", [5, 128, HK, D], bf16, kind="ExternalInput").ap()
    gb = nc.dram_tensor("gb", [128, sum(len(c) for c in CLASSES_OF), 64], f32,
                        kind="ExternalInput").ap()
    yp = nc.dram_tensor("yp", [128, KD, TPC], f32, kind="ExternalOutput").ap()
    yc = nc.dram_tensor("yc", [128, KD, NCLS * 64], f32, kind="ExternalOutput").ap()

    with tile.TileContext(nc) as tc, ExitStack() as ctx:
        wp = ctx.enter_context(tc.tile_pool(name="weights", bufs=1))
        xpool = ctx.enter_context(tc.tile_pool(name="xin", bufs=2))
        hpool = ctx.enter_context(tc.tile_pool(name="hmid", bufs=1))
        opool = ctx.enter_context(tc.tile_pool(name="out", bufs=1))
        cpool = ctx.enter_context(tc.tile_pool(name="cls", bufs=1))
        tpool = ctx.enter_context(tc.tile_pool(name="tmp", bufs=2))
        pspool = ctx.enter_context(tc.tile_pool(name="ps", bufs=8, space="PSUM"))

        # ---- DMA schedule ------------------------------------------------
        # critical path (sync HWDGE ring): a small first W1 chunk + first
        # token tile so matmuls start ASAP, then the rest of W1.
        w1t = wp.tile([128, MH, KD, 128], bf16)
        nc.sync.dma_start(w1t[:, 0:6], w1[:, 0:6])
        xt0 = xpool.tile([128, KD, TN], bf16, tag="xt", name="xt0")
        nc.sync.dma_start(xt0[:], xp[:, :, ts(0, TN)])
        b1t = wp.tile([128, MH], f32)
        nc.sync.dma_start(b1t[:], b1[:])
        b2t = wp.tile([128, KD], f32)
        nc.sync.dma_start(b2t[:], b2[:])

        # non-critical loads go on the scalar HWDGE ring, triggered between
        # gelu activations so they don't steal DMA bandwidth from W1/x0
        w2t = wp.tile([128, MH, D], bf16)
        xct = cpool.tile([128, KD, 768], bf16)
        a1t = cpool.tile([128, 5, KD, HS], bf16)
        a1bt = cpool.tile([128, 5, HK], f32)
        a2t = cpool.tile([128, 5, HK, D], bf16)
        gbt = cpool.tile([128, APP0[-1], 64], f32)
        h2 = [cpool.tile([128, HK, GW[c]], bf16, tag=f"h2_{c}", name=f"h2_{c}")
              for c in range(5)]
        outc = cpool.tile([128, KD, NCLS * 64], f32)

        # second token tile preallocated so its load can be deferred (with
        # bufs=2 the slot is free at kernel start, so an in-loop load would be
        # hoisted into the startup window and steal bandwidth from W1/x0)
        xt1 = xpool.tile([128, KD, TN], bf16, tag="xt", name="xt1")

        # one trigger per gelu slot: remaining W1 chunks stream just ahead of
        # the consuming psum groups, W2 by t0-L2 (~45us), cls inputs by ~220us
        _dl = {}
        _dl[(0, 13)] = lambda: nc.scalar.dma_start(xt1[:], xp[:, :, ts(1, TN)])
        for i, hm in enumerate(range(0, 18, 2)):
            _dl[(0, hm)] = lambda i=i: nc.scalar.dma_start(
                w1t[:, 6 + 2 * i:8 + 2 * i], w1[:, 6 + 2 * i:8 + 2 * i])
        for i in range(4):
            _dl[(0, 17 + 2 * i)] = lambda i=i: nc.scalar.dma_start(
                w2t[:, 6 * i:6 * (i + 1)], w2[:, 6 * i:6 * (i + 1)])
        _dl[(1, 0)] = lambda: nc.scalar.dma_start(xct[:], xc[:])
        for j in range(5):
            _dl[(1, 2 + 2 * j)] = lambda j=j: nc.scalar.dma_start(a1t[:, j], a1[j])
            _dl[(1, 14 + 2 * j)] = lambda j=j: nc.scalar.dma_start(a2t[:, j], a2[j])
        _dl[(1, 12)] = lambda: nc.scalar.dma_start(a1bt[:], a1b[:])
        _dl[(2, 0)] = lambda: nc.scalar.dma_start(gbt[:], gb[:])

        def _ins(x):
            return getattr(x, "ins", x)

        def deferred_loads(t, hm, act):
            fn = _dl.get((t, hm))
            if fn is not None:
                d = fn()
                # same-engine ordering edge: without it the scheduler hoists
                # the (dependency-free) trigger to kernel start, where its
                # transfer steals DMA bandwidth from the critical W1/x0 loads
                add_dep_helper(_ins(d), _ins(act), sync=False,
                               reason="defer bulk load behind gelu")

        # ---- patch MLP tile body ----------------------------------------
        def patch_tile(t, xt):
            ht = hpool.tile([128, MH, TN], bf16, tag="ht", name="ht")
            for hm in range(MH):
                pt = pspool.tile([128, TN], f32, tag="ps", name="pt")
                for k in range(KD):
                    nc.tensor.matmul(pt[:], w1t[:, hm, k, :], xt[:, k, :],
                                     start=(k == 0), stop=(k == KD - 1))
                act = nc.scalar.activation(ht[:, hm, :], pt[:], AF.Gelu,
                                           bias=b1t[:, hm, None])
                deferred_loads(t, hm, act)
            ot = opool.tile([128, KD, TN], f32, tag="ot", name="ot")
            for dm in range(KD):
                if t == NT - 1 and dm == KD - 1:
                    # final group split in two: first half's epilogue+store
                    # overlaps the second half's matmuls, shortening the tail
                    for hf in range(2):
                        HTN = TN // 2
                        pt = pspool.tile([128, TN], f32, tag="ps", name="pt")
                        po = pt[:, :HTN]
                        hsl = slice(hf * HTN, (hf + 1) * HTN)
                        for k in range(MH):
                            nc.tensor.matmul(po, w2t[:, k, ts(dm, 128)],
                                             ht[:, k, hsl],
                                             start=(k == 0), stop=(k == MH - 1))
                        nc.vector.tensor_scalar_add(ot[:, dm, hsl], po,
                                                    b2t[:, dm, None])
                        nc.sync.dma_start(
                            yp[:, dm, t * TN + hf * HTN:t * TN + (hf + 1) * HTN],
                            ot[:, dm, hsl])
                    continue
                pt = pspool.tile([128, TN], f32, tag="ps", name="pt")
                for k in range(MH):
                    nc.tensor.matmul(pt[:], w2t[:, k, ts(dm, 128)], ht[:, k, :],
                                     start=(k == 0), stop=(k == MH - 1))
                nc.vector.tensor_scalar_add(ot[:, dm, :], pt[:], b2t[:, dm, None])
                # per-dm store so the tail only waits for the last 256 KB
                nc.sync.dma_start(yp[:, dm, ts(t, TN)], ot[:, dm, :])

        def cls_phase():
            # first layer: per atom1 group, gelu results scattered into the
            # atom2-grouped h2 buffers
            for j in range(5):
                for hm in range(HK):
                    pt = pspool.tile([128, TN], f32, tag="ps", name="pt")
                    po = pt[:, :GW[j]]
                    for k in range(KD):
                        nc.tensor.matmul(po, a1t[:, j, k, ts(hm, 128)],
                                         xct[:, k, GOFF[j]:GOFF[j] + GW[j]],
                                         start=(k == 0), stop=(k == KD - 1))
                    for bidx, i in enumerate(CLASSES_OF[j]):
                        cdst = _other_atom(i, j)
                        pos = CLASSES_OF[cdst].index(i)
                        nc.scalar.activation(h2[cdst][:, hm, ts(pos, 64)],
                                             po[:, ts(bidx, 64)], AF.Gelu,
                                             bias=a1bt[:, j, hm, None])

            # second layer: per atom2 group, gated write/accumulate into outc
            outc4 = outc[:].rearrange("p d (l t) -> p d l t", t=64)
            for c in range(5):
                L = len(CLASSES_OF[c])
                gsl = gbt[:, APP0[c]:APP0[c] + L, :]
                for dm in range(KD):
                    pt = pspool.tile([128, TN], f32, tag="ps", name="pt")
                    po = pt[:, :GW[c]]
                    for hk in range(HK):
                        nc.tensor.matmul(po, a2t[:, c, hk, ts(dm, 128)],
                                         h2[c][:, hk, :],
                                         start=(hk == 0), stop=(hk == HK - 1))
                    pv = po.rearrange("p (l t) -> p l t", t=64)
                    if c < 3:
                        # classes 2c,2c+1 - first write of those columns
                        dst = outc4[:, dm, 2 * c:2 * c + 2, :]
                        nc.vector.tensor_tensor(dst, pv, gsl,
                                                mybir.AluOpType.mult)
                    else:
                        tmp = tpool.tile([128, 3, 64], f32, tag="ctmp",
                                         name="ctmp")
                        nc.vector.tensor_tensor(tmp[:, :L, :], pv, gsl,
                                                mybir.AluOpType.mult)
                        dst = outc4[:, dm, c - 3:NCLS:2, :]
                        nc.vector.tensor_add(dst, dst, tmp[:, :L, :])
            nc.sync.dma_start(yc[:], outc[:])

        # PE warmup: the HAM clock gate keeps the PE at 1.2 GHz until it has
        # been busy ~3.4us. The critical W1/x0 DMA takes ~8us, so run dummy
        # matmuls on zeros during that window - the real stream then starts
        # at 2.4 GHz instead of paying ~13 cold matmuls.
        warm = wp.tile([128, TN], bf16)
        nc.gpsimd.memset(warm[:], 0.0)
        wps = pspool.tile([128, TN], f32, tag="ps", name="warmps")
        for _ in range(14):
            nc.tensor.matmul(wps[:], warm[:, :128], warm[:], start=True, stop=True)

        # patch tiles 0..2, then cls (its epilogue hides under tile 3)
        for t in range(NT - 1):
            if t == 0:
                xt = xt0
            elif t == 1:
                xt = xt1  # load deferred to a t0 gelu slot
            else:
                xt = xpool.tile([128, KD, TN], bf16, tag="xt", name="xt")
                nc.sync.dma_start(xt[:], xp[:, :, ts(t, TN)])
            patch_tile(t, xt)
        cls_phase()
        xt = xpool.tile([128, KD, TN], bf16, tag="xt", name="xt")
        nc.sync.dma_start(xt[:], xp[:, :, ts(NT - 1, TN)])
        patch_tile(NT - 1, xt)

    nc.compile()
    _NC_CACHE["nc"] = nc
    return nc


# ---------------------------------------------------------------- host glue
def _bf(a):
    return np.ascontiguousarray(np.asarray(a), dtype=ml_dtypes.bfloat16)


def _f32(a):
    return np.ascontiguousarray(np.asarray(a), dtype=np.float32)


def _gates(x, G_W):
    """Mirror the reference's softmax/top-1/renorm gating in fp32 on host."""
    cls_tokens = np.asarray(x[:, :NCLS], dtype=np.float32)
    logits = np.einsum("bid,ide->bie", cls_tokens, np.asarray(G_W, np.float32))
    m = logits.max(-1, keepdims=True)
    e = np.exp(logits - m)
    gate = e / e.sum(-1, keepdims=True)
    thr = np.sort(gate, axis=-1)[..., -2]
    mask = (gate > thr[..., None]).astype(np.float32)
    g = gate * mask
    g = g / np.clip(g.sum(-1, keepdims=True), 1e-6, None)
    return g  # [B, NCLS, 2], entries exactly 0.0 or 1.0 (or 0/0 on exact ties)


def _shard_inputs(x, moe0_W1, moe0_b1, moe0_W2, moe0_b2, A1_W, A1_b, A2_W, A2_b, G_W):
    x = np.asarray(x, np.float32)

    # shared (replicated) tensors
    # [d, h] -> [p, hm, k, c] with d = k*128+p, h = hm*128+c
    w1v = _bf(np.asarray(moe0_W1, np.float32)).reshape(KD, 128, MH, 128)
    w1v = np.ascontiguousarray(w1v.transpose(1, 2, 0, 3))
    w2v = _bf(np.asarray(moe0_W2, np.float32)).reshape(MH, 128, D).transpose(1, 0, 2)
    w2v = np.ascontiguousarray(w2v)
    b1v = np.ascontiguousarray(_f32(moe0_b1).reshape(MH, 128).T)
    b2v = np.ascontiguousarray(_f32(moe0_b2).reshape(KD, 128).T)

    # stacked cls token groups -> [128, KD, 768] bf16
    xc_f = x[:, :NCLS, :]                                   # [B, 6, D]
    stacked = np.concatenate([xc_f[:, i, :] for j in range(5)
                              for i in CLASSES_OF[j]], axis=0)  # [768, D]
    xcv = _bf(stacked.T.reshape(KD, 128, 768).transpose(1, 0, 2))

    # gate multiplier broadcast [128, 12, 64]
    g = _gates(x, G_W)
    g_app = np.empty((APP0[-1], B), np.float32)
    for c in range(5):
        for pos, i in enumerate(CLASSES_OF[c]):
            e = [p[1] for p in PAIRS[i]].index(c)
            g_app[APP0[c] + pos] = g[:, i, e]
    gbv = np.ascontiguousarray(np.broadcast_to(g_app[None], (128,) + g_app.shape))

    A1_W = np.asarray(A1_W, np.float32)
    A2_W = np.asarray(A2_W, np.float32)
    A1_b = np.asarray(A1_b, np.float32)

    in_maps = []
    for core in range(NCORES):
        hs = slice(core * HS, (core + 1) * HS)
        # per-core patch tokens, transposed: [128, KD, TPC]
        xpc = x[core * BPC:(core + 1) * BPC, NCLS:, :].reshape(TPC, D)
        xpv = _bf(xpc.T.reshape(KD, 128, TPC).transpose(1, 0, 2))
        # atom slices
        a1v = _bf(A1_W[:, :, hs].reshape(5, KD, 128, HS).transpose(0, 2, 1, 3))
        a2v = _bf(A2_W[:, hs, :].reshape(5, HK, 128, D).transpose(0, 2, 1, 3))
        a1bv = np.ascontiguousarray(
            A1_b[:, hs].reshape(5, HK, 128).transpose(2, 0, 1))
        in_maps.append({
            "xp": xpv, "w1": w1v, "w2": w2v, "b1": b1v, "b2": b2v,
            "xc": xcv, "a1": a1v, "a1b": a1bv, "a2": a2v, "gb": gbv,
        })
    return in_maps, g


def _combine_outputs(results, g, A2_b):
    A2_b = np.asarray(A2_b, np.float32)
    out = np.empty((B, NCLS + PT, D), np.float32)
    for core in range(NCORES):
        ypv = results[core]["yp"]  # [128, KD, TPC]
        out[core * BPC:(core + 1) * BPC, NCLS:, :] = (
            ypv.transpose(2, 1, 0).reshape(BPC, PT, D))

    ycs = np.zeros((128, KD, NCLS, 64), np.float64)
    for core in range(NCORES):
        ycs += results[core]["yc"].reshape(128, KD, NCLS, 64)
    cls_out = ycs.transpose(3, 2, 1, 0).reshape(B, NCLS, D).astype(np.float32)

    # gated atom2 bias term (biases are added pre-gating in the reference)
    a2b_sel = np.stack([[A2_b[p[1]] for p in PAIRS[i]] for i in range(NCLS)])
    cls_out += np.einsum("bie,ied->bid", g, a2b_sel).astype(np.float32)
    out[:, :NCLS, :] = cls_out
    return out


def _run(inputs, trace=False, trace_kwargs=None):
    from concourse.bass_utils import run_bass_kernel_spmd

    nc = _build_nc()
    in_maps, g = _shard_inputs(
        inputs["x"], inputs["moe0_W1"], inputs["moe0_b1"], inputs["moe0_W2"],
        inputs["moe0_b2"], inputs["A1_W"], inputs["A1_b"], inputs["A2_W"],
        inputs["A2_b"], inputs["G_W"])
    res = run_bass_kernel_spmd(nc, in_maps, core_ids=list(range(NCORES)),
                               trace=trace, **(trace_kwargs or {}))
    out = _combine_outputs(res.results, g, inputs["A2_b"])
    return out, res


def kernel(**inputs) -> np.ndarray:
    out, _ = _run(inputs, trace=False)
    return out
